# revision 40
# baseline (speedup 1.0000x reference)
"""BoT tokenizer kernel for Trainium2 (Bass/Tile), 8-core data parallel.

All 25 output tokens are computed on the TensorEngine as bf16 matmuls with
an exact fp32 -> 2x bf16 mantissa split (8+8 = 16 bits):

    x = a0 + a1,  w = w0 + w1   (bf16 splits, exact by construction)
    x*w ~= a0*w0 + a0*w1 + a1*w0    (dropped a1*w1 term is ~2^-18 relative)

 - single-feature token k: K=5 matmul (3 cross products + 2 bias rows
   against a ones column)
 - fore token: 9 features -> K = 9*3 + 2 = 29
 - palm token: 7 features -> K = 7*3 + 2 = 23

The device writes the output as fp16 (26.2 MB/core instead of 52.4 MB
fp32) and the host upcasts to fp32; fp16 rounding adds ~2e-4 l2 relative
error, far inside the 2e-2 tolerance, and halves the output-DMA traffic
that bounds this kernel (~358 GB/s HBM per core -> ~73 us floor).

PSUM->SBUF conversion copies run in 2-token (2-bank) groups, 4 PSUM
buffers deep, alternating VectorE/ScalarE so the PSUM recycle pipeline
never serializes on one engine-buffer pair. Each 128-row chunk's fp16
output leaves as one 3.27 MB DMA, alternating the two HWDGE queues per
chunk (DMAs on one queue run strictly in FIFO order); the first/last
chunks are split into 3 pieces to shorten the pipeline fill and drain.
Inputs load as 4 wide DMAs over 3 queues (sync/scalar/gpsimd), with the
three singles slot tensors at partition bases 0/32/64 so their
transfers drain through different SDMA ports concurrently.
"""

import numpy as np

FORE_IDX = [0, 1, 2, 27, 28, 32, 33, 34, 38]
PALM_IDX = [4, 29, 30, 31, 35, 36, 37]
SINGLE_IDX = [3] + list(range(5, 27))

B = 8192
D = 512
T = 25
N_CORES = 8
B_LOC = B // N_CORES          # 1024 rows per core
CHUNK = 128
N_CHUNKS = B_LOC // CHUNK     # 8
ROW = T * D                   # 12800
NS = 23

KF = 9 * 3 + 2                # 29
KP = 7 * 3 + 2                # 23
KS = 5
# singles packed 3 per tile at 32-partition offsets (matmul base partition
# must be 32-aligned); tile partition extent 64+KS
S_TILES = [(a, min(a + 3, NS)) for a in range(0, NS, 3)]
S_STRIDE = 32

# PSUM copy groups: tokens [2g, 2g+2) (last group is token 24 alone)
N_GROUPS = 13

_prog_cache = {}


def _k_of_tok(t):
    return 0 if t == 1 else t - 2


def _build_program():
    import concourse.bacc as bacc
    import concourse.mybir as mybir
    import concourse.tile as tile
    from concourse.bass import ts

    f32 = mybir.dt.float32
    f16 = mybir.dt.float16
    bf16 = mybir.dt.bfloat16
    nc = bacc.Bacc("TRN2", target_bir_lowering=False, debug=False,
                   num_devices=N_CORES)

    NT = len(S_TILES)             # 8 singles tiles
    # inputs arrive as 4 tensors so each load is ONE wide DMA and the three
    # slot tensors land on distinct SDMA ports (partition bases 0/32/64):
    #   s{j} [KS, 12288]: singles lhsT (tile i at i*B_LOC) then singles rhs
    #                     (tile i at RS_O + i*D), for sensors k%3 == j
    #   fp [55, 1536]: fore/palm rhs at [0:D), fore/palm lhsT at [D:D+B_LOC)
    RS_O = NT * B_LOC             # 8192
    SW = RS_O + NT * D            # 12288
    s_d = [nc.dram_tensor(f"s{j}", [KS, SW], bf16, kind="ExternalInput")
           for j in range(3)]
    fp_d = nc.dram_tensor("fp", [S_STRIDE + KP, D + B_LOC], bf16,
                          kind="ExternalInput")
    out_d = nc.dram_tensor("out", [B_LOC, ROW], f16, kind="ExternalOutput")

    with tile.TileContext(nc) as tc:
        with (
            tc.tile_pool(name="cst", bufs=1) as cst,
            tc.tile_pool(name="op", bufs=1) as op,
            tc.tile_pool(name="pp", bufs=4, space="PSUM") as pp,
        ):
            # 4 input DMAs on 3 queues (HWDGE sync/scalar + gpsimd SWDGE);
            # slot tiles sit at partition bases 0/32/64 so their transfers
            # drain through different SDMA ports concurrently
            fp_s = cst.tile([S_STRIDE + KP, D + B_LOC], bf16)
            nc.sync.dma_start(out=fp_s[:], in_=fp_d[:])
            s_s = []
            for j, eng in enumerate((nc.sync, nc.scalar, nc.gpsimd)):
                t = cst.tile([S_STRIDE * j + KS, SW], bf16, name=f"s{j}_s")
                eng.dma_start(out=t[S_STRIDE * j:S_STRIDE * j + KS, :],
                              in_=s_d[j][:])
                s_s.append(t)

            for c in range(N_CHUNKS):
                o_t = op.tile([CHUNK, ROW], f16, tag="ot", bufs=5)
                for g in range(N_GROUPS):
                    t0, t1 = 2 * g, min(2 * g + 2, T)
                    p_t = pp.tile([CHUNK, 2 * D], f32)
                    for t in range(t0, t1):
                        if t == 0:
                            lhsT = fp_s[0:KF, D + c * CHUNK:
                                        D + (c + 1) * CHUNK]
                            rhs = fp_s[0:KF, 0:D]
                        elif t == 2:
                            lhsT = fp_s[S_STRIDE:S_STRIDE + KP,
                                        D + c * CHUNK:D + (c + 1) * CHUNK]
                            rhs = fp_s[S_STRIDE:S_STRIDE + KP, 0:D]
                        else:
                            k = _k_of_tok(t)
                            i, j = k // 3, k % 3
                            off = S_STRIDE * j
                            b0 = i * B_LOC + c * CHUNK
                            lhsT = s_s[j][off:off + KS, b0:b0 + CHUNK]
                            rhs = s_s[j][off:off + KS,
                                         RS_O + i * D:RS_O + (i + 1) * D]
                        nc.tensor.matmul(p_t[:, ts(t - t0, D)], lhsT, rhs,
                                         start=True, stop=True)
                    w = (t1 - t0) * D
                    dst = o_t[:, 2 * g * D:2 * g * D + w]
                    if g % 2 == 0:
                        nc.vector.tensor_copy(dst, p_t[:, :w])
                    else:
                        nc.scalar.copy(dst, p_t[:, :w])
                dma_eng = nc.sync if c % 2 == 0 else nc.scalar
                other = nc.scalar if c % 2 == 0 else nc.sync
                if c == 0 or c == N_CHUNKS - 1:
                    # finer pieces at the ends: shorter pipeline fill/drain
                    pieces = ((0, 6), (6, 12), (12, 18), (18, T))
                else:
                    # 2 pieces per chunk on alternating queues: the drain
                    # starts after 6 copy groups instead of all 13
                    pieces = ((0, 12), (12, T))
                for pi, (w0, w1) in enumerate(pieces):
                    eng = dma_eng if pi % 2 == 0 else other
                    eng.dma_start(
                        out=out_d[ts(c, CHUNK), w0 * D:w1 * D],
                        in_=o_t[:, w0 * D:w1 * D])

    nc.compile()
    return nc


def _split2(v):
    """Exact-ish fp32 -> (bf16, bf16) mantissa split: v ~= s0+s1."""
    import ml_dtypes
    bf = ml_dtypes.bfloat16
    v = np.asarray(v, np.float32)
    s0 = v.astype(bf)
    r1 = v - s0.astype(np.float32)
    s1 = r1.astype(bf)
    return s0, s1


def _lhs_rows(xcols):
    """lhsT rows for a feature block: a0,a0,a1 per feature.

    xcols: [B, F] fp32 -> [3F, B] bf16"""
    import ml_dtypes
    Bn, F = xcols.shape
    s0, s1 = _split2(xcols)              # each [B, F]
    out = np.empty((F, 3, Bn), dtype=ml_dtypes.bfloat16)
    out[:, 0, :] = s0.T
    out[:, 1, :] = s0.T
    out[:, 2, :] = s1.T
    return out.reshape(3 * F, Bn)


def _rhs_rows(wcols):
    """rhs rows for a feature block: w0,w1,w0 per feature.

    wcols: [F, D] fp32 -> [3F, D] bf16"""
    import ml_dtypes
    F, Dn = wcols.shape
    s0, s1 = _split2(wcols)
    out = np.empty((F, 3, Dn), dtype=ml_dtypes.bfloat16)
    out[:, 0, :] = s0
    out[:, 1, :] = s1
    out[:, 2, :] = s0
    return out.reshape(3 * F, Dn)


def _host_prep(x, Wf, bf_, Wp, bp, Ws, bs):
    import ml_dtypes
    bf16 = ml_dtypes.bfloat16

    ones2 = np.ones((2, B), dtype=bf16)

    def bias_rows(bias):
        b0, b1 = _split2(bias)           # [D] each
        return np.stack([b0, b1])        # [2, D]

    NT = len(S_TILES)
    # per-slot tensors: [KS, NT, B] lhs and [KS, NT, D] rhs for sensors
    # with k%3 == j; fore/palm: rhs [55, D], lhsT [55, B]
    ls = np.zeros((3, KS, NT, B), dtype=bf16)
    rs = np.zeros((3, KS, NT, D), dtype=bf16)
    rfp = np.zeros((S_STRIDE + KP, D), dtype=bf16)
    lfp = np.zeros((S_STRIDE + KP, B), dtype=bf16)
    lfp[0:KF] = np.concatenate([_lhs_rows(x[:, FORE_IDX]), ones2])
    rfp[0:KF] = np.concatenate([_rhs_rows(np.asarray(Wf.T)), bias_rows(bf_)])
    lfp[S_STRIDE:S_STRIDE + KP] = np.concatenate(
        [_lhs_rows(x[:, PALM_IDX]), ones2])
    rfp[S_STRIDE:S_STRIDE + KP] = np.concatenate(
        [_rhs_rows(np.asarray(Wp.T)), bias_rows(bp)])
    xs = x[:, SINGLE_IDX]                # [B, 23]
    for k in range(NS):
        i, j = k // 3, k % 3
        ls[j, 0:3, i] = _lhs_rows(xs[:, k:k + 1])
        ls[j, 3:KS, i] = ones2
        rs[j, 0:3, i] = _rhs_rows(Ws[k:k + 1])
        rs[j, 3:KS, i] = bias_rows(bs[k])
    return lfp, rfp, ls, rs


def kernel(x, Wf, bf, Wp, bp, Ws, bs, _trace=False, _spmd_kwargs=None):
    from concourse.bass_utils import run_bass_kernel_spmd

    x = np.asarray(x, np.float32)
    lfp, rfp, ls, rs = _host_prep(
        x, np.asarray(Wf, np.float32), np.asarray(bf, np.float32),
        np.asarray(Wp, np.float32), np.asarray(bp, np.float32),
        np.asarray(Ws, np.float32), np.asarray(bs, np.float32))

    if "nc" not in _prog_cache:
        _prog_cache["nc"] = _build_program()
    nc = _prog_cache["nc"]

    NT = len(S_TILES)
    rs2 = [rs[j].reshape(KS, NT * D) for j in range(3)]
    in_maps = []
    for i in range(N_CORES):
        sl = slice(i * B_LOC, (i + 1) * B_LOC)
        m = {"fp": np.ascontiguousarray(
            np.concatenate([rfp, lfp[:, sl]], axis=1))}
        for j in range(3):
            m[f"s{j}"] = np.ascontiguousarray(np.concatenate(
                [ls[j, :, :, sl].reshape(KS, NT * B_LOC), rs2[j]], axis=1))
        in_maps.append(m)

    kwargs = dict(_spmd_kwargs or {})
    res = run_bass_kernel_spmd(nc, in_maps, core_ids=list(range(N_CORES)),
                               trace=_trace, **kwargs)
    out = np.concatenate(
        [np.asarray(r["out"]).astype(np.float32) for r in res.results],
        axis=0)
    if _trace:
        kernel.last_results = res
    return out.reshape(B, T, D)
